# revision 1
# baseline (speedup 1.0000x reference)
"""ConvMultiheadAttention Trainium2 kernel (8 NeuronCores).

Sharding: core c = (batch b = c//2) x (head-group hg = c%2, 8 heads each).
Per core:
  - q/k/v conv1d projections (K=3, same pad) for this core's 512 output
    channels, expressed as PSUM-accumulated bf16 matmuls over x laid out
    [c_in partitions, L free].
  - attention with TRANSPOSED scores sT[j, i] (j on partitions) so the
    key-padding mask folds into the Exp activation's per-partition bias,
    and the softmax denominator comes for free from a ones-augmented
    AV matmul (row 64 of the [65, i] psum = column sums).
  - partial out-conv contracting over this core's 512 attention-output
    channels; the host sums the two partials per batch.
Host folds: attention scale + q-bias into q-conv weights/bias; k-bias is
dropped (constant per softmax row -> cancels); v-bias and o-bias are
applied on the host after the gather (attention rows sum to 1).
"""

import os
import numpy as np
import ml_dtypes

BF16 = ml_dtypes.bfloat16

B, L, D = 4, 1024, 1024
NH, HD = 16, 64
KW = 3
NCORES = 8
HALF = D // 2  # channels per core half (8 heads)
SCALE = HD ** -0.5
MASK_BIAS = -30000.0

_CACHE = {}


def _build_nc():
    import concourse.bass as bass  # noqa: F401
    import concourse.tile as tile
    from concourse import bacc, mybir

    f32 = mybir.dt.float32
    bf16 = mybir.dt.bfloat16
    Act = mybir.ActivationFunctionType

    nc = bacc.Bacc(
        "TRN2",
        target_bir_lowering=False,
        debug=False,
        enable_asserts=False,
        num_devices=NCORES,
    )

    # ---- DRAM I/O ----
    xq_d = nc.dram_tensor("xq", [8, 128, L], bf16, kind="ExternalInput").ap()
    xk_d = nc.dram_tensor("xk", [8, 128, L], bf16, kind="ExternalInput").ap()
    xv_d = nc.dram_tensor("xv", [8, 128, L], bf16, kind="ExternalInput").ap()
    wq_d = nc.dram_tensor("wq", [4, 128, KW, 8, 128], bf16, kind="ExternalInput").ap()
    wk_d = nc.dram_tensor("wk", [4, 128, KW, 8, 128], bf16, kind="ExternalInput").ap()
    wv_d = nc.dram_tensor("wv", [4, 128, KW, 8, 128], bf16, kind="ExternalInput").ap()
    wo_d = nc.dram_tensor("wo", [8, 128, KW, 4, 128], bf16, kind="ExternalInput").ap()
    qb_d = nc.dram_tensor("qb", [128, 4], f32, kind="ExternalInput").ap()
    jb_d = nc.dram_tensor("jb", [128, 8], f32, kind="ExternalInput").ap()
    out_d = nc.dram_tensor("out", [8, 128, L], f32, kind="ExternalOutput").ap()

    from concourse.masks import make_identity

    with tile.TileContext(nc) as tc:
        with (
            tc.tile_pool(name="singles", bufs=1) as singles,
            tc.tile_pool(name="wpool", bufs=3) as wpool,
            tc.tile_pool(name="qk", bufs=2) as qkpool,
            tc.tile_pool(name="vpool", bufs=2) as vpool,
            tc.tile_pool(name="ppool", bufs=2) as ppool,
            tc.tile_pool(name="outp", bufs=4) as outp,
            tc.tile_pool(name="smalls", bufs=4) as smalls,
            tc.tile_pool(name="convp", bufs=2, space="PSUM") as convp,
            tc.tile_pool(name="scorep", bufs=2, space="PSUM") as scorep,
            tc.tile_pool(name="avp", bufs=2, space="PSUM") as avp,
        ):
            # ---- constants / resident tiles ----
            ident = singles.tile([128, 128], bf16, tag="ident")
            make_identity(nc, ident)
            qb_s = singles.tile([128, 4], f32, tag="qb")
            nc.sync.dma_start(qb_s, qb_d)
            jb_s = singles.tile([128, 8], f32, tag="jb")
            nc.sync.dma_start(jb_s, jb_d)

            xq_s = singles.tile([128, 8, L], bf16, tag="xq")
            xk_s = singles.tile([128, 8, L], bf16, tag="xk")
            xv_s = singles.tile([128, 8, L], bf16, tag="xv")
            for cc in range(8):
                nc.sync.dma_start(xv_s[:, cc, :], xv_d[cc])
            for cc in range(8):
                nc.sync.dma_start(xq_s[:, cc, :], xq_d[cc])
            for cc in range(8):
                nc.sync.dma_start(xk_s[:, cc, :], xk_d[cc])

            # vT[j_part, j_chunk, head, 0:64] = v[h*64+d, j]; col 64 = ones
            vT = singles.tile([128, 8, 8, 65], bf16, tag="vT")
            for h in range(8):
                nc.vector.memset(vT[:, :, h, 64:65], 1.0)
            o_x = singles.tile([128, 4, L], bf16, tag="ox")

            def conv_mms(ps, w_t, x_t, n_ci, lh):
                """Accumulate conv-as-matmul into psum ps[:, 0:512] for
                output columns [lh*512, lh*512+512)."""
                # center tap first: full-width start=True write covers the
                # whole bank, so the edge taps' partial-width writes are
                # pure accumulations (uniform has_written state).
                korder = [1, 0, 2]
                first = True
                for k in korder:
                    for cc in range(n_ci):
                        lo = lh * 512 + k - 1
                        lhsT = w_t[:, k, cc, :]
                        if lo < 0:
                            rhs = x_t[:, cc, 0:511]
                            outap = ps[:, 1:512]
                        elif lo + 512 > L:
                            rhs = x_t[:, cc, lo:L]
                            outap = ps[:, 0 : L - lo]
                        else:
                            rhs = x_t[:, cc, lo : lo + 512]
                            outap = ps[:, 0:512]
                        nc.tensor.matmul(
                            outap,
                            lhsT,
                            rhs,
                            start=first,
                            stop=(k == korder[-1] and cc == n_ci - 1),
                        )
                        first = False

            # ---- V conv + transpose into vT ----
            for occ in range(4):
                wv_t = wpool.tile([128, KW, 8, 128], bf16, tag="w")
                nc.sync.dma_start(wv_t, wv_d[occ])
                v_t = vpool.tile([128, L], bf16, tag="v")
                for lh in range(2):
                    ps = convp.tile([128, 512], f32, tag="cp")
                    conv_mms(ps, wv_t, xv_s, 8, lh)
                    nc.vector.tensor_copy(v_t[:, lh * 512 : (lh + 1) * 512], ps)
                for lb in range(8):
                    tp = convp.tile([128, 128], bf16, tag="cp")
                    nc.tensor.transpose(tp, v_t[:, lb * 128 : (lb + 1) * 128], ident)
                    nc.vector.tensor_copy(vT[:, lb, 2 * occ, 0:64], tp[:, 0:64])
                    nc.vector.tensor_copy(vT[:, lb, 2 * occ + 1, 0:64], tp[:, 64:128])

            # ---- per head-pair: software-pipelined q/k conv + attention ----
            # Per pair t: scores+exp(t) -> q/k conv(t+1) [PE work that hides
            # exp(t) on ACT] -> AV+normalize(t).
            def qk_conv_units(t):
                """Return (q_t, k_t, units): four closures each emitting one
                conv psum-group (~5us of dense PE work) for pair t."""
                q_t = qkpool.tile([128, L], bf16, tag="q", name=f"q{t}")
                k_t = qkpool.tile([128, L], bf16, tag="k", name=f"k{t}")
                state = {}

                def unit(which, lh):
                    if which == "q" and lh == 0:
                        state["wq"] = wpool.tile([128, KW, 8, 128], bf16, tag="w",
                                                 name="wqt")
                        nc.sync.dma_start(state["wq"], wq_d[t])
                    if which == "k" and lh == 0:
                        state["wk"] = wpool.tile([128, KW, 8, 128], bf16, tag="w",
                                                 name="wkt")
                        nc.sync.dma_start(state["wk"], wk_d[t])
                    ps = convp.tile([128, 512], f32, tag="cp")
                    if which == "q":
                        conv_mms(ps, state["wq"], xq_s, 8, lh)
                        nc.vector.tensor_scalar_add(
                            q_t[:, lh * 512 : (lh + 1) * 512], ps,
                            qb_s[:, t : t + 1])
                    else:
                        conv_mms(ps, state["wk"], xk_s, 8, lh)
                        nc.vector.tensor_copy(
                            k_t[:, lh * 512 : (lh + 1) * 512], ps)

                units = [lambda w=w, lh=lh: unit(w, lh)
                         for w in ("q", "k") for lh in range(2)]
                return q_t, k_t, units

            def qk_conv(t):
                q_t, k_t, units = qk_conv_units(t)
                for u in units:
                    u()
                return q_t, k_t

            _ablate = os.environ.get("KERNEL_ABLATE", "")
            q_t, k_t = qk_conv(0)
            if _ablate == "noattn":
                nc.vector.memset(o_x, 0.01)
                for t in range(1, 4):
                    q_t, k_t = qk_conv(t)
            for t in range(4 if _ablate != "noattn" else 0):
                # next pair's conv psum-groups, interleaved between score
                # jc-groups below so PE stays busy while ACT runs exp(t)
                if t < 3:
                    nq_t, nk_t, conv_units = qk_conv_units(t + 1)
                else:
                    conv_units = []
                # scores + exp for both heads; adjacent matmuls of the two
                # heads hit disjoint PE row groups (base 0 / 64) and overlap.
                p_pair = []
                for jc in range(8):
                    sps_pair = [scorep.tile([128, L], f32, tag="score",
                                            name=f"sps{hh2}")
                                for hh2 in range(2)]
                    if jc == 0:
                        p_pair = [ppool.tile([128, 8, L], bf16, tag="p",
                                             name=f"p{hh2}")
                                  for hh2 in range(2)]
                    for ih in range(2):
                        for hh in range(2):
                            base = hh * 64
                            nc.tensor.matmul(
                                sps_pair[hh][:, ih * 512 : (ih + 1) * 512],
                                k_t[base : base + 64, jc * 128 : (jc + 1) * 128],
                                q_t[base : base + 64, ih * 512 : (ih + 1) * 512],
                                start=True,
                                stop=True,
                            )
                    for hh in range(2):
                        nc.scalar.activation(
                            p_pair[hh][:, jc, :], sps_pair[hh], Act.Exp,
                            bias=jb_s[:, jc : jc + 1],
                        )
                    # one conv psum-group (~5us dense PE) after every other
                    # jc-group: fills the PE stall while ACT drains exp(t)
                    if jc % 2 == 1 and conv_units:
                        conv_units.pop(0)()
                # AV + normalize for both heads
                for hh in range(2):
                    h = 2 * t + hh
                    base = hh * 64
                    for ih in range(2):
                        avps = avp.tile([65, 512], f32, tag="av")
                        for jc in range(8):
                            nc.tensor.matmul(
                                avps,
                                vT[:, jc, h, :],
                                p_pair[hh][:, jc, ih * 512 : (ih + 1) * 512],
                                start=(jc == 0),
                                stop=(jc == 7),
                            )
                        r_t = smalls.tile([1, 512], f32, tag="r")
                        nc.vector.reciprocal(r_t, avps[64:65, :])
                        bc_t = smalls.tile([64, 512], f32, tag="bc")
                        nc.gpsimd.partition_broadcast(bc_t, r_t)
                        dst = o_x[base : base + 64, t, ih * 512 : (ih + 1) * 512]
                        if hh == 0:
                            nc.vector.tensor_mul(dst, avps[0:64, :], bc_t)
                        else:
                            tmp = smalls.tile([64, 512], bf16, tag="tmp")
                            nc.vector.tensor_mul(tmp, avps[0:64, :], bc_t)
                            nc.sync.dma_start(dst, tmp)
                if t < 3:
                    q_t, k_t = nq_t, nk_t

            # ---- out conv (partial over this core's 512 input channels) ----
            for occ in range(8 if _ablate != "nooconv" else 0):
                wo_t = wpool.tile([128, KW, 4, 128], bf16, tag="w")
                nc.sync.dma_start(wo_t, wo_d[occ])
                for lh in range(2):
                    ps = convp.tile([128, 512], f32, tag="cp")
                    conv_mms(ps, wo_t, o_x, 4, lh)
                    o_t = outp.tile([128, 512], f32, tag="osb")
                    nc.vector.tensor_copy(o_t, ps)
                    nc.sync.dma_start(out_d[occ, :, lh * 512 : (lh + 1) * 512], o_t)

    nc.compile()
    return nc


def _get_nc():
    if "nc" not in _CACHE:
        _CACHE["nc"] = _build_nc()
    return _CACHE["nc"]


def _prep_inputs(query, key, value, key_padding_mask, attn_mask,
                 q_w, q_b, k_w, k_b, v_w, v_b, o_w, o_b):
    """Build the 8 per-core input maps (host-side shard + layout)."""
    query = np.asarray(query, np.float32)
    key = np.asarray(key, np.float32)
    value = np.asarray(value, np.float32)
    kpm = np.asarray(key_padding_mask)
    attn_mask = np.asarray(attn_mask, np.float32)
    q_w = np.asarray(q_w, np.float32); q_b = np.asarray(q_b, np.float32)
    k_w = np.asarray(k_w, np.float32)
    v_w = np.asarray(v_w, np.float32)
    o_w = np.asarray(o_w, np.float32); o_b = np.asarray(o_b, np.float32)

    # attn_mask must be constant across query rows to fold into the key bias
    if not np.all(attn_mask == attn_mask[0:1, :]):
        raise NotImplementedError("attn_mask varying over query index unsupported")
    am_row = attn_mask[0]

    def conv_w_layout(w, occ, n_ci):
        # w: [C_out_part, C_in_part, KW] -> [occ, p(ci), k, cc, m(c_out)]
        co, ci, _ = w.shape
        arr = w.reshape(occ, 128, n_ci, 128, KW).transpose(0, 3, 4, 2, 1)
        return np.ascontiguousarray(arr).astype(BF16)

    wq_h, wk_h, wv_h, wo_h, qb_h = [], [], [], [], []
    for hg in range(2):
        sl = slice(hg * HALF, (hg + 1) * HALF)
        wq_h.append(conv_w_layout(q_w[sl] * SCALE, 4, 8))
        wk_h.append(conv_w_layout(k_w[sl], 4, 8))
        wv_h.append(conv_w_layout(v_w[sl], 4, 8))
        # out conv: contract over this half's input channels
        wo_h.append(conv_w_layout(o_w[:, sl, :], 8, 4))
        qb_h.append(np.ascontiguousarray(
            (q_b[sl] * SCALE).reshape(4, 128).T).astype(np.float32))

    xq_b, xk_b, xv_b, jb_b = [], [], [], []
    for b in range(B):
        xq_b.append(np.ascontiguousarray(query[b].T).reshape(8, 128, L).astype(BF16))
        xk_b.append(np.ascontiguousarray(key[b].T).reshape(8, 128, L).astype(BF16))
        xv_b.append(np.ascontiguousarray(value[b].T).reshape(8, 128, L).astype(BF16))
        jb = np.where(kpm[b], MASK_BIAS, 0.0).astype(np.float32) + am_row
        jb_b.append(np.ascontiguousarray(jb.reshape(8, 128).T).astype(np.float32))

    in_maps = []
    for c in range(NCORES):
        b, hg = c // 2, c % 2
        in_maps.append({
            "xq": xq_b[b], "xk": xk_b[b], "xv": xv_b[b],
            "wq": wq_h[hg], "wk": wk_h[hg], "wv": wv_h[hg], "wo": wo_h[hg],
            "qb": qb_h[hg], "jb": jb_b[b],
        })
    return in_maps, (o_w, np.asarray(v_b, np.float32), o_b)


def _postprocess(parts, extras):
    """parts: list of 8 arrays [8,128,L] f32 -> full output [B, L, D] f32."""
    o_w, v_b, o_b = extras
    # v-bias contribution through the out conv (attention rows sum to 1):
    # interior columns see all 3 taps, edge columns lose one.
    a_full = o_w.sum(axis=2) @ v_b            # [D]
    a_l0 = a_full - o_w[:, :, 0] @ v_b        # l = 0 loses tap k=0
    a_lL = a_full - o_w[:, :, 2] @ v_b        # l = L-1 loses tap k=2
    out = np.empty((B, L, D), np.float32)
    for b in range(B):
        tot = (parts[2 * b] + parts[2 * b + 1]).reshape(D, L)
        tot = tot + o_b[:, None] + a_full[:, None]
        tot[:, 0] += a_l0 - a_full
        tot[:, -1] += a_lL - a_full
        out[b] = tot.T
    return out


def _run(in_maps, trace=False, **kw):
    from concourse import bass_utils
    nc = _get_nc()
    try:
        res = bass_utils.run_bass_kernel_spmd(
            nc, in_maps, core_ids=list(range(NCORES)), trace=trace, **kw)
    except ModuleNotFoundError:
        # NTFF profiling hook unavailable (axon client without axon.trn);
        # rerun without trace.
        res = bass_utils.run_bass_kernel_spmd(
            nc, in_maps, core_ids=list(range(NCORES)), trace=False, **kw)
    return res


def kernel(**inputs) -> np.ndarray:
    in_maps, extras = _prep_inputs(**inputs)
    res = _run(in_maps, trace=bool(int(os.environ.get("KERNEL_TRACE", "0"))))
    parts = [res.results[c]["out"] for c in range(NCORES)]
    out = _postprocess(parts, extras)
    if res.exec_time_ns is not None:
        print(f"HW exec time: {res.exec_time_ns} ns")
    return out



# revision 13
# speedup vs baseline: 1.0514x; 1.0514x over previous
"""ConvMultiheadAttention Trainium2 kernel (8 NeuronCores).

Sharding: core c = (batch b = c//2) x (head-group hg = c%2, 8 heads each).

Per core, all four conv1d(K=3) projections run as Winograd F(2,3) in a
parity-split column layout: L positions are stored as (parity, t) with
i_p = parity*512 + t, so the Winograd input transform is four packed
+-1-offset bf16 adds per input chunk (DVE 4x mode) and every conv matmul
is a full-width [128ci x 128co] x [128 x 512tile] PSUM accumulation --
2/3 the tensor-engine cycles of the direct 3-tap form.  M-quads live in
a [128, 3, 512] psum tile (slot0 reused for m3); the output transform
y_even = m0+m1+m2 / y_odd = m1-m2-m3 is split across Pool (partial sums)
and DVE (final merge, q-bias folded via scalar_tensor_tensor).

Attention is the baseline scheme in parity order: transposed scores
sT[j_p, i_p] so the key-padding mask folds into the Exp bias (host
reorders the bias to parity order), ones-augmented AV matmul for the
softmax denominator.  The v-conv occ slices 1..3 and the q/k conv slices
for pair t+1 are interleaved into pair t's score/exp phase to keep PE
busy while ACT drains the exps.

Host folds: attention scale + q-bias into q-conv weights/bias; k-bias
dropped (cancels in softmax); v-bias and o-bias applied on the host
after the gather (attention rows sum to 1).  Output ships bf16 in
parity layout; the host interleaves and sums the two head-group
partials per batch.
"""

import os
import numpy as np
import ml_dtypes

BF16 = ml_dtypes.bfloat16

B, L, D = 4, 1024, 1024
NH, HD = 16, 64
KW = 3
NCORES = 8
HALF = D // 2  # channels per core half (8 heads)
T = L // 2     # winograd tiles per row
SCALE = HD ** -0.5
MASK_BIAS = -30000.0

_CACHE = {}


def _build_nc():
    import concourse.bass as bass  # noqa: F401
    import concourse.tile as tile
    from concourse import bacc, mybir

    f32 = mybir.dt.float32
    bf16 = mybir.dt.bfloat16
    Act = mybir.ActivationFunctionType
    Alu = mybir.AluOpType

    nc = bacc.Bacc(
        "TRN2",
        target_bir_lowering=False,
        debug=False,
        enable_asserts=False,
        num_devices=NCORES,
    )

    # ---- DRAM I/O ----
    # x: per input-channel chunk cc, parity-extended columns:
    #   [:, 0, :] = [x_even(512), 0pad]; [:, 1, :] = [0pad, x_odd(512)]
    xq_d = nc.dram_tensor("xq", [8, 128, 2, T + 1], bf16, kind="ExternalInput").ap()
    xk_d = nc.dram_tensor("xk", [8, 128, 2, T + 1], bf16, kind="ExternalInput").ap()
    xv_d = nc.dram_tensor("xv", [8, 128, 2, T + 1], bf16, kind="ExternalInput").ap()
    # winograd-transformed weights: [occ/t, ci_p, j, cc, co]
    wq_d = nc.dram_tensor("wq", [4, 128, 4, 8, 128], bf16, kind="ExternalInput").ap()
    wk_d = nc.dram_tensor("wk", [4, 128, 4, 8, 128], bf16, kind="ExternalInput").ap()
    wv_d = nc.dram_tensor("wv", [4, 128, 4, 8, 128], bf16, kind="ExternalInput").ap()
    wo_d = nc.dram_tensor("wo", [8, 128, 4, 4, 128], bf16, kind="ExternalInput").ap()
    qb_d = nc.dram_tensor("qb", [128, 4], f32, kind="ExternalInput").ap()
    jb_d = nc.dram_tensor("jb", [128, 8], f32, kind="ExternalInput").ap()
    out_d = nc.dram_tensor("out", [8, 128, 2, T], bf16, kind="ExternalOutput").ap()

    from concourse.masks import make_identity

    with tile.TileContext(nc) as tc:
        with (
            tc.tile_pool(name="singles", bufs=1) as singles,
            tc.tile_pool(name="xep", bufs=2) as xep,
            tc.tile_pool(name="wpool", bufs=2) as wpool,
            tc.tile_pool(name="vtp", bufs=2) as vtp,
            tc.tile_pool(name="qk", bufs=2) as qkpool,
            tc.tile_pool(name="ppool", bufs=1) as ppool,
            tc.tile_pool(name="ytmp", bufs=3) as ytp,
            tc.tile_pool(name="outp", bufs=2) as outp,
            tc.tile_pool(name="smalls", bufs=1) as smalls,
            tc.tile_pool(name="convp", bufs=2, space="PSUM") as convp,
            tc.tile_pool(name="sp", bufs=2, space="PSUM") as sp,
        ):
            # ---- constants / resident tiles ----
            qb_s = singles.tile([128, 4], f32, tag="qb")
            nc.sync.dma_start(qb_s, qb_d)
            jb_s = singles.tile([128, 8], f32, tag="jb")
            nc.sync.dma_start(jb_s, jb_d)
            ident = singles.tile([128, 128], bf16, tag="ident")
            make_identity(nc, ident)

            # vT[j_part, jc, head, 0:64] = v[h*64+d, j]; col 64 = ones
            vT = singles.tile([128, 8, 8, 65], bf16, tag="vT")
            for h in range(8):
                nc.vector.memset(vT[:, :, h, 64:65], 1.0)
            # attention output, parity-extended for the out-conv transform:
            # [ch, t_pair, 0, :] = [even(512), 0]; [ch, t_pair, 1, :] = [0, odd]
            o_x = singles.tile([128, 4, 2, T + 1], bf16, tag="ox")
            nc.vector.memset(o_x[:, :, 0, T : T + 1], 0.0)
            nc.vector.memset(o_x[:, :, 1, 0:1], 0.0)
            # winograd input transforms (v/q/k live through pair 3)
            v_v = singles.tile([128, 4, 8, T], bf16, tag="vv")
            v_q = singles.tile([128, 4, 8, T], bf16, tag="vq")
            v_k = singles.tile([128, 4, 8, T], bf16, tag="vk")
            v_ox = singles.tile([128, 4, 4, T], bf16, tag="vox")

            def load_and_transform(x_d, vdst, n_ci=8):
                """DMA x chunks and emit the F(2,3) input transform:
                V0 = xo[t-1]-xo[t], V1 = xe+xo, V2 = xo-xe, V3 = xe-xe[t+1]."""
                for cc in range(n_ci):
                    xt = xep.tile([128, 2, T + 1], bf16, tag="xe", name=f"xe{cc}")
                    nc.sync.dma_start(xt, x_d[cc])
                    xe_ = xt[:, 0, :]
                    xo_ = xt[:, 1, :]
                    nc.vector.tensor_sub(vdst[:, 0, cc, :], xo_[:, 0:T], xo_[:, 1 : T + 1])
                    nc.vector.tensor_add(vdst[:, 1, cc, :], xe_[:, 0:T], xo_[:, 1 : T + 1])
                    nc.vector.tensor_sub(vdst[:, 2, cc, :], xo_[:, 1 : T + 1], xe_[:, 0:T])
                    nc.vector.tensor_sub(vdst[:, 3, cc, :], xe_[:, 0:T], xe_[:, 1 : T + 1])

            # DMA order: xv first (v conv starts the kernel), weights
            # interleaved so the first matmul can start ~4us in.  The
            # load_and_transform calls are emitted between prologue conv
            # units so DVE-FIFO order matches data arrival (no head-of-line
            # blocking of the conv output transforms behind x transforms).
            load_and_transform(xv_d, v_v)
            w_v0 = wpool.tile([128, 4, 8, 128], bf16, tag="w", name="wv0")
            nc.sync.dma_start(w_v0, wv_d[0])

            JSLOT = [0, 1, 2, 0]  # m3 reuses slot 0 after y_even consumed m0

            def conv_chains(w_t, vsrc, dst, n_ci, qb_col=None):
                """Return 4 closures, chain j = the 8(4)-matmul contraction for
                M_j plus the staged output transform.  dst: [128, 2, T] bf16."""
                state = {}

                def chain(j):
                    if j == 0:
                        state["quad"] = convp.tile([128, 3, 512], f32, tag="quad",
                                                   name="quad")
                    quad = state["quad"]
                    for cc in range(n_ci):
                        nc.tensor.matmul(
                            quad[:, JSLOT[j], :],
                            w_t[:, j, cc, :],
                            vsrc[:, j, cc, :],
                            start=(cc == 0),
                            stop=(cc == n_ci - 1),
                        )
                    # Engines can read only ONE psum operand per op, and
                    # GPSIMD none at all: stage m1 to SBUF on ACT (slack
                    # there), and ship U2 NEGATED from the host so both y
                    # terms are adds of m1_sb with one psum operand:
                    #   y_even = m0+m1+m2 = (m1_sb+m0) - m2'
                    #   y_odd  = m1-m2-m3 = (m1_sb+m2') - m3
                    if j == 1:
                        m1_sb = ytp.tile([128, 512], bf16, tag="yt", name="m1")
                        nc.scalar.copy(m1_sb, quad[:, 1, :])
                        state["m1"] = m1_sb
                    if j == 2:
                        t1 = ytp.tile([128, 512], bf16, tag="yt", name="t1")
                        t2 = ytp.tile([128, 512], bf16, tag="yt", name="t2")
                        nc.vector.tensor_add(t1, state["m1"], quad[:, 0, :])
                        nc.vector.tensor_add(t2, state["m1"], quad[:, 2, :])
                        state["t1"], state["t2"] = t1, t2
                    if j == 3:
                        t1, t2 = state["t1"], state["t2"]
                        if qb_col is not None:
                            nc.vector.scalar_tensor_tensor(
                                dst[:, 0, :], t1, qb_col, quad[:, 2, :],
                                op0=Alu.add, op1=Alu.subtract)
                            nc.vector.scalar_tensor_tensor(
                                dst[:, 1, :], t2, qb_col, quad[:, 0, :],
                                op0=Alu.add, op1=Alu.subtract)
                        else:
                            nc.vector.tensor_sub(dst[:, 0, :], t1, quad[:, 2, :])
                            nc.vector.tensor_sub(dst[:, 1, :], t2, quad[:, 0, :])

                return [lambda j=j: chain(j) for j in range(4)]

            def v_conv_unit(occ, w_t):
                """4 chains + transposes into vT for v-conv occ slice."""
                v_t = vtp.tile([128, 2, T], bf16, tag="vt")
                chains = conv_chains(w_t, v_v, v_t, 8)

                def finish():
                    for jc in range(8):
                        tp = sp.tile([128, 128], bf16, tag="sp", name="tp")
                        nc.tensor.transpose(
                            tp, v_t[:, jc // 4, (jc % 4) * 128 : (jc % 4 + 1) * 128],
                            ident)
                        nc.vector.tensor_copy(vT[:, jc, 2 * occ, 0:64], tp[:, 0:64])
                        nc.vector.tensor_copy(vT[:, jc, 2 * occ + 1, 0:64],
                                              tp[:, 64:128])

                return chains + [finish]

            def q_conv_unit(t, wq_t=None):
                q_t = qkpool.tile([128, 2, T], bf16, tag="q", name=f"q{t}")
                if wq_t is None:
                    wq_t = wpool.tile([128, 4, 8, 128], bf16, tag="w", name="wqt")
                    nc.sync.dma_start(wq_t, wq_d[t])
                return q_t, conv_chains(wq_t, v_q, q_t, 8,
                                        qb_col=qb_s[:, t : t + 1])

            def k_conv_unit(t, wk_t=None):
                k_t = qkpool.tile([128, 2, T], bf16, tag="k", name=f"k{t}")
                if wk_t is None:
                    wk_t = wpool.tile([128, 4, 8, 128], bf16, tag="w", name="wkt")
                    nc.sync.dma_start(wk_t, wk_d[t])
                return k_t, conv_chains(wk_t, v_k, k_t, 8)

            def qk_conv_units(t):
                q_t, qch = q_conv_unit(t)
                k_t, kch = k_conv_unit(t)
                return q_t, k_t, qch + kch

            # ---- prologue: v occ0, q0, k0 (x transforms emitted between
            # units so each engine FIFO tracks data arrival) ----
            v0_units = v_conv_unit(0, w_v0)
            for u in v0_units[:4]:
                u()
            load_and_transform(xq_d, v_q)
            w_q0 = wpool.tile([128, 4, 8, 128], bf16, tag="w", name="wq0")
            nc.sync.dma_start(w_q0, wq_d[0])
            v0_units[4]()  # transposes + vT copies
            q_t, qch0 = q_conv_unit(0, wq_t=w_q0)
            for u in qch0:
                u()
            load_and_transform(xk_d, v_k)
            w_k0 = wpool.tile([128, 4, 8, 128], bf16, tag="w", name="wk0")
            nc.sync.dma_start(w_k0, wk_d[0])
            k_t, kch0 = k_conv_unit(0, wk_t=w_k0)
            for u in kch0:
                u()

            def load_vw(occ):
                w = wpool.tile([128, 4, 8, 128], bf16, tag="w", name=f"wv{occ}")
                nc.sync.dma_start(w, wv_d[occ])
                return w

            # interleave schedule: per pair t, one unit-chain per jc group.
            # qk(t+1) must finish within pair t; v occ o before AV(o).
            # Weight tiles are allocated in consumption order (wpool bufs=2
            # rotation: each DMA waits on the tile two requests back).
            def pair_fill_units(t):
                if t == 0:
                    nq, nk, ch = qk_conv_units(1)
                    return nq, nk, ch + v_conv_unit(1, load_vw(1))
                if t == 1:
                    nq, nk, ch = qk_conv_units(2)
                    return nq, nk, ch
                if t == 2:
                    nq, nk, ch = qk_conv_units(3)
                    return nq, nk, ch + v_conv_unit(2, load_vw(2))
                return None, None, v_conv_unit(3, load_vw(3))

            def ox_transform(t):
                """F(2,3) input transform of the attention output for pair t."""
                oe = o_x[:, t, 0, :]
                oo = o_x[:, t, 1, :]
                nc.vector.tensor_sub(v_ox[:, 0, t, :], oo[:, 0:T], oo[:, 1 : T + 1])
                nc.vector.tensor_add(v_ox[:, 1, t, :], oe[:, 0:T], oo[:, 1 : T + 1])
                nc.vector.tensor_sub(v_ox[:, 2, t, :], oo[:, 1 : T + 1], oe[:, 0:T])
                nc.vector.tensor_sub(v_ox[:, 3, t, :], oe[:, 0:T], oe[:, 1 : T + 1])

            for t in range(4):
                nq_t, nk_t, fill = pair_fill_units(t)
                fill = list(fill)
                p_pair = [ppool.tile([128, 8, L], bf16, tag=f"p{hh}", name=f"p{hh}")
                          for hh in range(2)]
                for jc in range(8):
                    for hh in range(2):
                        base = hh * 64
                        for ih in range(2):
                            sps = sp.tile([128, 512], f32, tag="sp", name="sps")
                            nc.tensor.matmul(
                                sps,
                                k_t[base : base + 64, jc // 4,
                                    (jc % 4) * 128 : (jc % 4 + 1) * 128],
                                q_t[base : base + 64, ih, :],
                                start=True,
                                stop=True,
                            )
                            nc.scalar.activation(
                                p_pair[hh][:, jc, ih * 512 : (ih + 1) * 512],
                                sps, Act.Exp, bias=jb_s[:, jc : jc + 1],
                            )
                    if fill:
                        fill.pop(0)()
                for u in fill:
                    u()
                # prefetch out-conv weights during the last pair
                if t == 3:
                    wo_t0 = wpool.tile([128, 4, 4, 128], bf16, tag="w", name="wo0")
                    nc.sync.dma_start(wo_t0, wo_d[0])
                # AV + normalize
                for hh in range(2):
                    h = 2 * t + hh
                    base = hh * 64
                    for ih in range(2):
                        avps = sp.tile([65, 512], f32, tag="sp", name="av")
                        for jc in range(8):
                            nc.tensor.matmul(
                                avps,
                                vT[:, jc, h, :],
                                p_pair[hh][:, jc, ih * 512 : (ih + 1) * 512],
                                start=(jc == 0),
                                stop=(jc == 7),
                            )
                        r_t = smalls.tile([1, 512], f32, tag="r")
                        nc.vector.reciprocal(r_t, avps[64:65, :])
                        bc_t = smalls.tile([64, 512], f32, tag="bc")
                        nc.gpsimd.partition_broadcast(bc_t, r_t)
                        # parity ih: even -> cols 0:T of slot 0; odd -> 1:T+1
                        dst = o_x[base : base + 64, t, ih, ih : ih + T]
                        if hh == 0:
                            nc.vector.tensor_mul(dst, avps[0:64, :], bc_t)
                        else:
                            tmp = smalls.tile([64, 512], bf16, tag="tmp")
                            nc.vector.tensor_mul(tmp, avps[0:64, :], bc_t)
                            nc.sync.dma_start(dst, tmp)
                ox_transform(t)
                if t < 3:
                    q_t, k_t = nq_t, nk_t

            # ---- out conv (partial over this core's 512 input channels) ----
            wo_t = wo_t0
            for occ in range(8):
                if occ < 7:
                    wo_n = wpool.tile([128, 4, 4, 128], bf16, tag="w",
                                      name=f"wo{occ + 1}")
                    nc.sync.dma_start(wo_n, wo_d[occ + 1])
                o_t = outp.tile([128, 2, T], bf16, tag="osb")
                for ch in conv_chains(wo_t, v_ox, o_t, 4):
                    ch()
                nc.sync.dma_start(out_d[occ], o_t)
                if occ < 7:
                    wo_t = wo_n

    nc.compile()
    return nc


def _get_nc():
    if "nc" not in _CACHE:
        _CACHE["nc"] = _build_nc()
    return _CACHE["nc"]


def _prep_inputs(query, key, value, key_padding_mask, attn_mask,
                 q_w, q_b, k_w, k_b, v_w, v_b, o_w, o_b):
    """Build the 8 per-core input maps (host-side shard + layout)."""
    query = np.asarray(query, np.float32)
    key = np.asarray(key, np.float32)
    value = np.asarray(value, np.float32)
    kpm = np.asarray(key_padding_mask)
    attn_mask = np.asarray(attn_mask, np.float32)
    q_w = np.asarray(q_w, np.float32); q_b = np.asarray(q_b, np.float32)
    k_w = np.asarray(k_w, np.float32)
    v_w = np.asarray(v_w, np.float32)
    o_w = np.asarray(o_w, np.float32); o_b = np.asarray(o_b, np.float32)

    # attn_mask must be constant across query rows to fold into the key bias
    if not np.all(attn_mask == attn_mask[0:1, :]):
        raise NotImplementedError("attn_mask varying over query index unsupported")
    am_row = attn_mask[0]

    def parity_ext(xT):
        # xT: [D, L] -> [8, 128, 2, T+1] bf16, parity-extended
        arr = np.zeros((8, 128, 2, T + 1), np.float32)
        xr = xT.reshape(8, 128, L)
        arr[:, :, 0, 0:T] = xr[:, :, 0::2]
        arr[:, :, 1, 1 : T + 1] = xr[:, :, 1::2]
        return arr.astype(BF16)

    def conv_w_layout(w, n_oc, n_ci):
        # w: [Cout, Cin, 3] -> winograd U, layout [n_oc, 128ci_p, 4j, n_ci, 128co]
        u = np.stack([
            w[:, :, 0],
            (w[:, :, 0] + w[:, :, 1] + w[:, :, 2]) * 0.5,
            (w[:, :, 1] - w[:, :, 0] - w[:, :, 2]) * 0.5,  # NEGATED U2
            w[:, :, 2],
        ])  # [4, Cout, Cin]
        u = u.reshape(4, n_oc, 128, n_ci, 128)      # [j, oc, co, cc, ci_p]
        u = u.transpose(1, 4, 0, 3, 2)              # [oc, ci_p, j, cc, co]
        return np.ascontiguousarray(u).astype(BF16)

    wq_h, wk_h, wv_h, wo_h, qb_h = [], [], [], [], []
    for hg in range(2):
        sl = slice(hg * HALF, (hg + 1) * HALF)
        wq_h.append(conv_w_layout(q_w[sl] * SCALE, 4, 8))
        wk_h.append(conv_w_layout(k_w[sl], 4, 8))
        wv_h.append(conv_w_layout(v_w[sl], 4, 8))
        # out conv: contract over this half's input channels
        wo_h.append(conv_w_layout(o_w[:, sl, :], 8, 4))
        qb_h.append(np.ascontiguousarray(
            (q_b[sl] * SCALE).reshape(4, 128).T).astype(np.float32))

    xq_b, xk_b, xv_b, jb_b = [], [], [], []
    for b in range(B):
        xq_b.append(parity_ext(query[b].T))
        xk_b.append(parity_ext(key[b].T))
        xv_b.append(parity_ext(value[b].T))
        jb = np.where(kpm[b], MASK_BIAS, 0.0).astype(np.float32) + am_row
        jb_p = np.concatenate([jb[0::2], jb[1::2]])  # parity order
        jb_b.append(np.ascontiguousarray(jb_p.reshape(8, 128).T).astype(np.float32))

    in_maps = []
    for c in range(NCORES):
        b, hg = c // 2, c % 2
        in_maps.append({
            "xq": xq_b[b], "xk": xk_b[b], "xv": xv_b[b],
            "wq": wq_h[hg], "wk": wk_h[hg], "wv": wv_h[hg], "wo": wo_h[hg],
            "qb": qb_h[hg], "jb": jb_b[b],
        })
    return in_maps, (o_w, np.asarray(v_b, np.float32), o_b)


def _postprocess(parts, extras):
    """parts: list of 8 arrays [8,128,2,T] bf16 -> full output [B, L, D] f32."""
    o_w, v_b, o_b = extras
    # v-bias contribution through the out conv (attention rows sum to 1):
    # interior columns see all 3 taps, edge columns lose one.
    a_full = o_w.sum(axis=2) @ v_b            # [D]
    a_l0 = a_full - o_w[:, :, 0] @ v_b        # l = 0 loses tap k=0
    a_lL = a_full - o_w[:, :, 2] @ v_b        # l = L-1 loses tap k=2
    out = np.empty((B, L, D), np.float32)
    for b in range(B):
        s = parts[2 * b].astype(np.float32) + parts[2 * b + 1].astype(np.float32)
        tot = np.empty((D, L), np.float32)
        totv = tot.reshape(8, 128, L)
        totv[:, :, 0::2] = s[:, :, 0, :]
        totv[:, :, 1::2] = s[:, :, 1, :]
        tot = tot + o_b[:, None] + a_full[:, None]
        tot[:, 0] += a_l0 - a_full
        tot[:, -1] += a_lL - a_full
        out[b] = tot.T
    return out


def _run(in_maps, trace=False, **kw):
    from concourse import bass_utils
    nc = _get_nc()
    try:
        res = bass_utils.run_bass_kernel_spmd(
            nc, in_maps, core_ids=list(range(NCORES)), trace=trace, **kw)
    except ModuleNotFoundError:
        # NTFF profiling hook unavailable (axon client without axon.trn);
        # rerun without trace.
        res = bass_utils.run_bass_kernel_spmd(
            nc, in_maps, core_ids=list(range(NCORES)), trace=False, **kw)
    return res


def kernel(**inputs) -> np.ndarray:
    in_maps, extras = _prep_inputs(**inputs)
    res = _run(in_maps, trace=bool(int(os.environ.get("KERNEL_TRACE", "0"))))
    parts = [res.results[c]["out"] for c in range(NCORES)]
    out = _postprocess(parts, extras)
    if res.exec_time_ns is not None:
        print(f"HW exec time: {res.exec_time_ns} ns")
    return out
